# revision 7
# baseline (speedup 1.0000x reference)
"""Single-head self-attention (B=4, S=2048, D=1024) on 8 trn2 NeuronCores.

Sharding: core c -> (batch b = c//2, query half h = c%2); data-parallel over
batch, sequence-parallel over queries within a batch. Each core receives its
batch's x in both layouts (x^T d-major for projections/scores, x native
t-major for the attention-weighted contraction) with its own seq-half first
(softmax is invariant to key permutation). The host gather is then a pure
concatenation of [1024, 1024] output blocks.

Per-core algorithm (no K^T and no V are ever materialized):
  Q^T = Wq-proj of the core's 1024 queries (+bq)        [1024, 1024]
  G   = Wk @ Q^T        (K projection applied on the small Q side)
  scores^T[t, s] = sum_d xT[d, t] G[d, s]   (K bias cancels in softmax;
                   max-subtraction skipped: scores ~ N(0, 0.33))
  expP = exp(scores^T / 32); E = sum of expP tiles (DVE chain)
  l[s] via one N=2 matmul per query tile against a ones vector
  H^T[d, s] = sum_t x[t, d] expP[t, s]      (attn contracts x first)
  out[s, j] = (sum_d H^T[d, s] Wv[d, j]) / l[s] + bv[j]
This is the zero-duplication floor of 15.05 GFLOP/core (1/8 of the
network's total work) with no inter-core communication.

v2 (all-bf16 operands, fp32 accumulate): halves HBM traffic 42.6 -> 19 MB
(x loaded once per layout, everything resident in SBUF), spreads coalesced
input DMAs across the sync/gpsimd/vector/scalar queues so the PE never
waits on loads, and orders phases Q,G,S0,S1,H0,H1,O0,O1 (both 512-query
s-blocks resident) so every inter-phase latency is covered by independent
matmuls. The 1/l softmax normalization rides the scalar engine's
per-partition activation scale; only the bv add stays on DVE.
"""

import os
import sys
import types

import numpy as np

B, S, D = 4, 2048, 1024
HALF = S // 2  # 1024 queries per core
SCALE = 1.0 / 32.0  # 1/sqrt(D)
NC = 8
DC = D // 128  # 8 d-chunks
TT = S // 128  # 16 key tiles
SBLK = 512  # queries per s-block
NSB = HALF // SBLK  # 2 s-blocks

_CACHED_NC = None
LAST_RESULT = None  # BassKernelResults of the most recent run (for test.py)


def _ensure_axon_ntff_hook():
    """bass_utils' trace path needs antenv.axon_hooks; this image's antenv
    lacks it. Install a shim backed by trn_agent_boot's ctypes hook so
    BASS_TRACE=1 profiling works. No-op if already present/unavailable."""
    try:
        import antenv.axon_hooks  # noqa: F401

        return
    except ImportError:
        pass
    try:
        from trn_agent_boot.trn_boot import _ntff_profile_via_ctypes

        hook = _ntff_profile_via_ctypes("/opt/axon/libaxon_pjrt.so")
    except Exception:
        hook = None
    mod = types.ModuleType("antenv.axon_hooks")
    mod.get_axon_ntff_profile_hook = lambda: hook
    mod.set_axon_ntff_profile_hook = lambda h: None
    sys.modules["antenv.axon_hooks"] = mod


def build_kernel(tc, xt, xn, wq, wk, wv, bq, bv, out):
    import concourse.bass as bass
    from concourse import mybir

    nc = tc.nc
    F32 = mybir.dt.float32
    F32R = mybir.dt.float32r
    BF16 = mybir.dt.bfloat16
    Identity = mybir.ActivationFunctionType.Identity
    Copy = mybir.ActivationFunctionType.Copy
    Exp = mybir.ActivationFunctionType.Exp

    xt_r = xt.rearrange("(c p) t -> p c t", p=128)  # [128, 8, 2048]
    xn_r = xn.rearrange("(tc p) d -> p tc d", p=128)  # [128, 16, 1024]
    out_r = out.rearrange("(su p) j -> su p j", p=128)  # [8, 128, 1024]

    with tc.tile_pool(name="persist", bufs=1) as persist:
        xT = persist.tile([128, DC, S], BF16)
        xN = persist.tile([128, TT, D], BF16)
        wv_sb = persist.tile([128, DC, D], BF16)
        G = persist.tile([128, DC, HALF], BF16)
        bv_bc = persist.tile([128, D], F32)
        bq_sb = persist.tile([128, DC], F32)
        ones_f = persist.tile([128, 2], F32)
        ones_t = persist.tile([128, 2], BF16)
        ones_r = persist.tile([128, 2], F32R)

        # ---- Input DMA schedule: everything issued up front, coalesced ----
        # (>=1KB contiguous runs per partition), spread across the three
        # DMA-capable queues (sync/gpsimd/scalar) so no single queue gates
        # the PE. Arrival order matches consumption: xT t-block 0 + leading
        # wq j-slices first (phase A), then the rest.
        # sync: all of xT (tb0 lo-half first), later the output blocks.
        nc.sync.dma_start(xT[:, 0:4, 0:512], xt_r[:, 0:4, 0:512])
        # gpsimd: xT tb0(c4-7) + wq j-slices 1,3, then xn, then wv.
        nc.gpsimd.dma_start(xT[:, 4:8, 0:512], xt_r[:, 4:8, 0:512])
        for tb in range(1, 4):
            nc.sync.dma_start(
                xT[:, :, tb * 512 : (tb + 1) * 512],
                xt_r[:, :, tb * 512 : (tb + 1) * 512],
            )
        bv_bcast_ap = bass.AP(
            tensor=bv.tensor, offset=bv.offset, ap=[[0, 128]] + list(bv.ap)
        )
        nc.scalar.dma_start(bq_sb, bq)
        nc.scalar.dma_start(bv_bc, bv_bcast_ap)
        nc.vector.memset(ones_f, 1.0)
        nc.vector.tensor_copy(ones_t, ones_f)
        nc.vector.tensor_copy(ones_r, ones_f)

        with (
            tc.tile_pool(name="pa", bufs=1) as pa,
            tc.tile_pool(name="psa", bufs=2, space="PSUM") as psa,
            tc.tile_pool(name="psw", bufs=1, space="PSUM") as psw,
        ):
            wq_sb = pa.tile([128, DC, D], BF16)
            wk_sb = pa.tile([128, DC, D], BF16)
            qT = pa.tile([128, DC, HALF], BF16)
            # wq j-slices alternate scalar/gpsimd so slice k lands before the
            # qc=2k..2k+1 chains consume it; wk follows on scalar (needed a
            # full Q-phase later).
            nc.scalar.dma_start(wq_sb[:, :, 0:256], wq[:, :, 0:256])
            nc.gpsimd.dma_start(wq_sb[:, :, 256:512], wq[:, :, 256:512])
            nc.scalar.dma_start(wq_sb[:, :, 512:768], wq[:, :, 512:768])
            nc.gpsimd.dma_start(wq_sb[:, :, 768:1024], wq[:, :, 768:1024])
            nc.scalar.dma_start(wk_sb[:, :, 0:512], wk[:, :, 0:512])
            nc.scalar.dma_start(wk_sb[:, :, 512:1024], wk[:, :, 512:1024])
            # gpsimd then streams the phase-B inputs: xn blocks + wv.
            for tb in range(4):
                nc.gpsimd.dma_start(
                    xN[:, tb * 4 : (tb + 1) * 4, :], xn_r[:, tb * 4 : (tb + 1) * 4, :]
                )
            nc.gpsimd.dma_start(wv_sb, wv)

            # PE warmup: tiny input-independent matmuls run during the input
            # DMA wait so the HAM clock gate is at 2.4 GHz when real work
            # arrives (it otherwise starts cold at 1.2 GHz).
            warm = psw.tile([2, 2], F32, tag="warm")
            for _ in range(96):
                nc.tensor.matmul(warm, ones_t, ones_t, start=True, stop=True)

            # ---- Phase A: Q^T then G = Wk @ Q^T --------------------------
            # sblk-outer so the first chains need only xT t-block 0 and the
            # leading wq j-slices.
            for sblk in range(NSB):
                for qc in range(DC):
                    qpsum = psa.tile([128, SBLK], F32, tag="qpsum")
                    for c in range(DC):
                        nc.tensor.matmul(
                            qpsum,
                            wq_sb[:, c, qc * 128 : (qc + 1) * 128],
                            xT[:, c, sblk * SBLK : (sblk + 1) * SBLK],
                            start=(c == 0),
                            stop=(c == DC - 1),
                        )
                    nc.scalar.activation(
                        qT[:, qc, sblk * SBLK : (sblk + 1) * SBLK],
                        qpsum,
                        Identity,
                        bias=bq_sb[:, qc : qc + 1],
                    )
            # G[d, s] = sum_j Wk[d, j] qT[j, s]  (wk passed j-major = Wk.T)
            for sblk in range(NSB):
                for gc in range(DC):
                    gpsum = psa.tile([128, SBLK], F32, tag="gpsum")
                    for jc in range(DC):
                        nc.tensor.matmul(
                            gpsum,
                            wk_sb[:, jc, gc * 128 : (gc + 1) * 128],
                            qT[:, jc, sblk * SBLK : (sblk + 1) * SBLK],
                            start=(jc == 0),
                            stop=(jc == DC - 1),
                        )
                    nc.scalar.activation(
                        G[:, gc, sblk * SBLK : (sblk + 1) * SBLK], gpsum, Copy
                    )

        # ---- Phase B: S0 S1 (scores+exp), H0 H1, O0 O1 -------------------
        # Both s-blocks stay resident so every phase boundary is covered by
        # the other block's independent matmuls.
        with (
            tc.tile_pool(name="pb", bufs=1) as pb,
            tc.tile_pool(name="pb_o", bufs=2) as pbo,
            tc.tile_pool(name="pb_m", bufs=2) as pbm,
            tc.tile_pool(name="psb_s", bufs=2, space="PSUM") as psbs,
            tc.tile_pool(name="psb_h", bufs=2, space="PSUM") as psbh,
            tc.tile_pool(name="psb_o", bufs=2, space="PSUM") as psbo,
            tc.tile_pool(name="psb_l", bufs=2, space="PSUM") as psbl,
        ):
            expP0 = pb.tile([128, TT, SBLK], BF16)
            expP1 = pb.tile([128, TT, SBLK], BF16)
            E_t0 = pb.tile([128, SBLK], F32R)
            E_t1 = pb.tile([128, SBLK], F32R)
            H0 = pb.tile([128, DC, SBLK], BF16)
            H1 = pb.tile([128, DC, SBLK], BF16)
            expP = [expP0, expP1]
            E_t = [E_t0, E_t1]
            H = [H0, H1]

            for sb in range(NSB):
                for tt in range(TT):
                    spsum = psbs.tile([128, SBLK], F32, tag="spsum")
                    for c in range(DC):
                        nc.tensor.matmul(
                            spsum,
                            xT[:, c, tt * 128 : (tt + 1) * 128],
                            G[:, c, sb * SBLK : (sb + 1) * SBLK],
                            start=(c == 0),
                            stop=(c == DC - 1),
                        )
                    nc.scalar.activation(expP[sb][:, tt, :], spsum, Exp, scale=SCALE)
                    if tt == 1:
                        nc.vector.tensor_add(
                            E_t[sb], expP[sb][:, 0, :], expP[sb][:, 1, :]
                        )
                    elif tt > 1:
                        nc.vector.tensor_add(E_t[sb], E_t[sb], expP[sb][:, tt, :])

            # H^T[d, s] = sum_t x[t, d] expP[t, s]
            for sb in range(NSB):
                for dc in range(DC):
                    hpsum = psbh.tile([128, SBLK], F32, tag="hpsum")
                    for tt in range(TT):
                        nc.tensor.matmul(
                            hpsum,
                            xN[:, tt, dc * 128 : (dc + 1) * 128],
                            expP[sb][:, tt, :],
                            start=(tt == 0),
                            stop=(tt == TT - 1),
                        )
                    nc.scalar.activation(H[sb][:, dc, :], hpsum, Copy)

            # out[s, j] = (sum_d H^T[d, s] Wv[d, j]) / l[s] + bv[j]
            for sb in range(NSB):
                for su in range(SBLK // 128):
                    s0 = su * 128
                    lpsum = psbl.tile([128, 2], F32, tag="lpsum")
                    nc.tensor.matmul(
                        lpsum, E_t[sb][:, s0 : s0 + 128], ones_r, start=True, stop=True
                    )
                    recip = pbm.tile([128, 1], F32, tag="recip")
                    nc.vector.reciprocal(recip, lpsum[:, 0:1])
                    for jb in range(2):
                        opsum = psbo.tile([128, 512], F32, tag="opsum")
                        for dc in range(DC):
                            nc.tensor.matmul(
                                opsum,
                                H[sb][:, dc, s0 : s0 + 128],
                                wv_sb[:, dc, jb * 512 : (jb + 1) * 512],
                                start=(dc == 0),
                                stop=(dc == DC - 1),
                            )
                        o_sb = pbo.tile([128, 512], F32, tag="o_sb")
                        nc.scalar.activation(o_sb, opsum, Identity, scale=recip)
                        nc.vector.tensor_add(
                            o_sb, o_sb, bv_bc[:, jb * 512 : (jb + 1) * 512]
                        )
                        nc.sync.dma_start(
                            out_r[sb * (SBLK // 128) + su][
                                :, jb * 512 : (jb + 1) * 512
                            ],
                            o_sb,
                        )


def build_nc():
    global _CACHED_NC
    if _CACHED_NC is not None:
        return _CACHED_NC
    import concourse.tile as tile
    from concourse import bacc, mybir

    F32 = mybir.dt.float32
    BF16 = mybir.dt.bfloat16
    nc = bacc.Bacc("TRN2", target_bir_lowering=False, debug=False)
    xt = nc.dram_tensor("xt", [D, S], BF16, kind="ExternalInput").ap()
    xn = nc.dram_tensor("xn", [S, D], BF16, kind="ExternalInput").ap()
    wq = nc.dram_tensor("wq", [128, DC, D], BF16, kind="ExternalInput").ap()
    wk = nc.dram_tensor("wk", [128, DC, D], BF16, kind="ExternalInput").ap()
    wv = nc.dram_tensor("wv", [128, DC, D], BF16, kind="ExternalInput").ap()
    bq = nc.dram_tensor("bq", [128, DC], F32, kind="ExternalInput").ap()
    bv = nc.dram_tensor("bv", [D], F32, kind="ExternalInput").ap()
    out = nc.dram_tensor("out", [HALF, D], F32, kind="ExternalOutput").ap()

    with tile.TileContext(nc) as tc:
        build_kernel(tc, xt, xn, wq, wk, wv, bq, bv, out)
    nc.compile()
    _CACHED_NC = nc
    return nc


def _shard_inputs(x, Wq, bq, Wk, bk, Wv, bv):
    """Host-side prep: per-core permuted x^T + relaid-out bf16 weights."""
    import ml_dtypes

    bf16 = ml_dtypes.bfloat16
    wq_r = np.ascontiguousarray(
        Wq.reshape(DC, 128, D).transpose(1, 0, 2).astype(bf16)
    )
    wk_r = np.ascontiguousarray(
        Wk.T.reshape(DC, 128, D).transpose(1, 0, 2).astype(bf16)
    )
    wv_r = np.ascontiguousarray(
        Wv.reshape(DC, 128, D).transpose(1, 0, 2).astype(bf16)
    )
    bq_r = np.ascontiguousarray(bq.reshape(DC, 128).T)
    bv_c = np.ascontiguousarray(bv)

    in_maps = []
    for c in range(NC):
        b, h = divmod(c, 2)
        xb = x[b]
        if h:
            xb = np.concatenate([xb[HALF:], xb[:HALF]], axis=0)
        xb16 = xb.astype(bf16)
        xt = np.ascontiguousarray(xb16.T)  # [D, S], own queries first
        xn = np.ascontiguousarray(xb16)  # [S, D], same permutation
        in_maps.append(
            {
                "xt": xt,
                "xn": xn,
                "wq": wq_r,
                "wk": wk_r,
                "wv": wv_r,
                "bq": bq_r,
                "bv": bv_c,
            }
        )
    return in_maps


def kernel(x, Wq, bq, Wk, bk, Wv, bv):
    global LAST_RESULT
    _ensure_axon_ntff_hook()
    from concourse import bass_utils

    x = np.asarray(x, dtype=np.float32)
    args = [np.asarray(a, dtype=np.float32) for a in (Wq, bq, Wk, bk, Wv, bv)]
    nc = build_nc()
    in_maps = _shard_inputs(x, *args)
    res = bass_utils.run_bass_kernel_spmd(nc, in_maps, core_ids=list(range(NC)))
    LAST_RESULT = res
    out = np.empty((B, S, D), dtype=np.float32)
    for c in range(NC):
        b, h = divmod(c, 2)
        out[b, h * HALF : (h + 1) * HALF, :] = res.results[c]["out"]
    return out


if __name__ == "__main__":
    rng = np.random.default_rng(0)
    init = 1.0 / 32.0
    x = rng.standard_normal((B, S, D), dtype=np.float32)
    mk = lambda *s: rng.uniform(-init, init, s).astype(np.float32)
    o = kernel(x, mk(D, D), mk(D), mk(D, D), mk(D), mk(D, D), mk(D))
    print("out", o.shape, o.dtype, float(np.abs(o).max()))
